# revision 4
# baseline (speedup 1.0000x reference)
"""Trainium2 Bass kernel for batched attention.

Problem: b=16 batches of softmax(Q K^T / sqrt(128)) V with n=m=2048, d=dv=128,
fp32 inputs/outputs.  Sharding: batch dim across 8 NeuronCores (2 per core).

v2 design (vs v1 baseline at ~99us):
  - Q^T/K^T via DMA XBAR transpose (SBUF f16 -> SBUF f16), freeing the PE of
    64 transposes and the DVE of the psum->sbuf copies.
  - exp split across TWO engines: ACT does exact exp (scale fused); DVE does
    a Schraudolph-style int16 exp approximation in a single tensor_scalar:
      m = round/trunc(S * 1024*log2(e)/T + (15360 - C_ADJ)); bitcast(m) ~ p
    Mean bias of the approximation cancels in softmax (num/denom); the
    mantissa-linearization ripple (~2%) survives at sqrt(share) weight.
  - MM2 per batch in two half-passes: A = chunks 0..7 accumulated in PSUM
    while the same batch's exp stream is still running, parked to SBUF (f16);
    B = chunks 8..15 + a PE identity-matmul merge of the parked partial into
    the same PSUM accumulation group. Ones column appended to V gives the
    softmax denominator in column 128.
  - Normalize on ACT (Copy activation, per-partition reciprocal scale from
    DVE), store via sync HWDGE.
"""

import math

import numpy as np

B = 16
N_CORES = 8
B_LOC = B // N_CORES  # 2 batches per core
N = 2048
M = 2048
D = 128
NT = N // 128  # 16
MT = M // 128  # 16
TEMP = 11.313708498984761
INV_TEMP = 1.0 / TEMP

# Schraudolph constants for the DVE exp path (fp16 bit pattern via int16).
ALPHA = 1024.0 * math.log2(math.e) / TEMP
C_ADJ = 38.0
BETA = 15360.0 - C_ADJ

# exp unit (c, h) -> DVE if table says so; 13/32 units (~40%) on DVE.
N_UNITS = 2 * MT  # 32 per batch, production order i = 2*c + h
DVE_SHARE_NUM = 13
DVE_UNIT = [((i * DVE_SHARE_NUM) % N_UNITS) < DVE_SHARE_NUM for i in range(N_UNITS)]

_CACHE = {}


def _build():
    import concourse.bacc as bacc
    import concourse.mybir as mybir
    import concourse.tile as tile
    from concourse.masks import make_identity

    f32 = mybir.dt.float32
    f16 = mybir.dt.float16
    i16 = mybir.dt.int16

    nc = bacc.Bacc("TRN2", target_bir_lowering=False, debug=False,
                   num_devices=N_CORES)
    q_dram = nc.dram_tensor("queries", [B_LOC, N, D], f32, kind="ExternalInput")
    k_dram = nc.dram_tensor("keys", [B_LOC, M, D], f32, kind="ExternalInput")
    v_dram = nc.dram_tensor("values", [B_LOC, M, D], f32, kind="ExternalInput")
    o_dram = nc.dram_tensor("out", [B_LOC, N, D], f32, kind="ExternalOutput")

    with tile.TileContext(nc) as tc:
        with (
            tc.tile_pool(name="const", bufs=1) as const_pool,
            tc.tile_pool(name="nat", bufs=4) as nat_pool,
            tc.tile_pool(name="qT", bufs=2) as qT_pool,
            tc.tile_pool(name="kT", bufs=2) as kT_pool,
            tc.tile_pool(name="vo", bufs=2) as vo_pool,
            tc.tile_pool(name="pT", bufs=26) as pT_pool,
            tc.tile_pool(name="oall", bufs=2) as o_pool,
            tc.tile_pool(name="pa", bufs=20) as pa_pool,
            tc.tile_pool(name="small", bufs=6) as small_pool,
            tc.tile_pool(name="psS", bufs=3, space="PSUM") as psS_pool,
            tc.tile_pool(name="psO", bufs=2, space="PSUM") as psO_pool,
        ):
            ident = const_pool.tile([128, 128], f16)
            make_identity(nc, ident[:])

            # ---------------- input DMA (all cast-loads on the Pool queue) --
            nats = []   # (q_nat, k_nat) per batch
            vos = []
            for b in range(B_LOC):
                q_nat = nat_pool.tile([128, N], f16, tag="nat")
                k_nat = nat_pool.tile([128, M], f16, tag="nat")
                nats.append((q_nat, k_nat))
                vo = vo_pool.tile([128, MT * 129], f16)
                vos.append(vo)

            def load_half(dst, srcd, b, g):
                cs = slice(g * 8, (g + 1) * 8)
                nc.gpsimd.dma_start(
                    dst[:].rearrange("p (c d) -> p c d", d=128)[:, cs],
                    srcd[b].rearrange("(c p) d -> p c d", p=128)[:, cs])

            def load_v(b):
                vo = vos[b]
                nc.gpsimd.dma_start(
                    vo[:].rearrange("p (c w) -> p c w", w=129)[:, :, 0:128],
                    v_dram[b].rearrange("(c p) d -> p c d", p=128))
                nc.gpsimd.memset(
                    vo[:].rearrange("p (c w) -> p c w", w=129)[:, :, 128:129],
                    1.0)

            # batch 0: K halves first (stationaries), then Q halves, then V
            load_half(nats[0][1], k_dram, 0, 0)
            load_half(nats[0][0], q_dram, 0, 0)
            load_half(nats[0][1], k_dram, 0, 1)
            load_half(nats[0][0], q_dram, 0, 1)
            load_v(0)
            load_half(nats[1][1], k_dram, 1, 0)
            load_half(nats[1][0], q_dram, 1, 0)
            load_half(nats[1][1], k_dram, 1, 1)
            load_half(nats[1][0], q_dram, 1, 1)
            load_v(1)

            # ---------------- XBAR transposes on sync HWDGE -----------------
            qTs, kTs = [], []
            for b in range(B_LOC):
                qT = qT_pool.tile([128, N], f16)
                kT = kT_pool.tile([128, M], f16)
                qTs.append(qT)
                kTs.append(kT)

            def xpose(dst, src, h):
                cols = slice(h * 1024, (h + 1) * 1024)
                nc.sync.dma_start_transpose(
                    dst[:, cols].rearrange("k (c n) -> k c n", n=128),
                    src[:, cols])

            for b in range(B_LOC):
                xpose(kTs[b], nats[b][1], 0)
                xpose(qTs[b], nats[b][0], 0)
                xpose(kTs[b], nats[b][1], 1)
                xpose(qTs[b], nats[b][0], 1)

            # ---------------- compute -------------------------------------
            o_alls = [o_pool.tile([128, NT * 128], f32, name="oall",
                                  tag="oall") for _ in range(B_LOC)]
            pas = {}   # (b, t) -> parked first-half MM2 partial (f16)
            pTs = {b: [] for b in range(B_LOC)}

            def mm1_exp(b, c):
                """MM1 chunk c (both halves) + its two exp units."""
                pT = pT_pool.tile([128, N], f16, tag="pT")
                pTs[b].append(pT)
                kT, qT = kTs[b], qTs[b]
                for h in range(2):
                    psS = psS_pool.tile([128, 1024], f32, tag="psS")
                    for j in range(2):
                        nc.tensor.matmul(
                            psS[:, j * 512:(j + 1) * 512],
                            kT[:, c * 128:(c + 1) * 128],
                            qT[:, h * 1024 + j * 512:h * 1024 + (j + 1) * 512],
                            start=True, stop=True)
                    cols = slice(h * 1024, (h + 1) * 1024)
                    if DVE_UNIT[2 * c + h]:
                        nc.vector.tensor_scalar(
                            pT[:, cols].bitcast(i16), psS[:],
                            ALPHA, BETA,
                            mybir.AluOpType.mult, mybir.AluOpType.add)
                    else:
                        nc.scalar.activation(
                            pT[:, cols], psS[:],
                            mybir.ActivationFunctionType.Exp, scale=INV_TEMP)

            def mm2_a(b, t):
                """First-half MM2 (chunks 0..7) -> parked SBUF partial."""
                psA = psO_pool.tile([128, 129], f32, tag="psO")
                vo = vos[b]
                for c in range(8):
                    nc.tensor.matmul(
                        psA[:],
                        pTs[b][c][:, t * 128:(t + 1) * 128],
                        vo[:, c * 129:(c + 1) * 129],
                        start=(c == 0), stop=(c == 7))
                pa = pa_pool.tile([128, 129], f16, tag="pa")
                pas[(b, t)] = pa
                nc.vector.tensor_copy(pa[:], psA[:])

            def mm2_b(b, t):
                """Second-half MM2 + identity-matmul merge of the parked
                partial, then reciprocal (DVE) + normalize (ACT) + store."""
                psO = psO_pool.tile([128, 129], f32, tag="psO")
                vo = vos[b]
                for c in range(8, MT):
                    nc.tensor.matmul(
                        psO[:],
                        pTs[b][c][:, t * 128:(t + 1) * 128],
                        vo[:, c * 129:(c + 1) * 129],
                        start=(c == 8), stop=False)
                nc.tensor.matmul(
                    psO[:], ident[:], pas[(b, t)][:],
                    start=False, stop=True)
                recip = small_pool.tile([128, 1], f32, tag="recip")
                nc.vector.reciprocal(recip[:], psO[:, 128:129])
                nc.scalar.activation(
                    o_alls[b][:, t * 128:(t + 1) * 128], psO[:, 0:128],
                    mybir.ActivationFunctionType.Copy, scale=recip[:])
                if t % 4 == 3:
                    g = t // 4
                    cs = slice(g * 4, (g + 1) * 4)
                    nc.sync.dma_start(
                        o_dram[b].rearrange("(c p) d -> p c d", p=128)[:, cs],
                        o_alls[b][:].rearrange("p (c d) -> p c d", d=128)[:, cs])

            # batch 0: MM1/exp stream with own first-half MM2 interleaved
            for c in range(MT):
                mm1_exp(0, c)
                if c >= 8:
                    mm2_a(0, 2 * (c - 8))
                    mm2_a(0, 2 * (c - 8) + 1)

            # batch 1 stream: MM1/exp + b0 second-half MM2 + own first-half
            for c in range(MT):
                mm1_exp(1, c)
                mm2_b(0, c)
                if c >= 8:
                    mm2_a(1, 2 * (c - 8))
                    mm2_a(1, 2 * (c - 8) + 1)

            # drain: batch 1 second-half MM2
            for t in range(NT):
                mm2_b(1, t)

    nc.compile()
    return nc


def _get_nc():
    if "nc" not in _CACHE:
        _CACHE["nc"] = _build()
    return _CACHE["nc"]


def _ensure_ntff_hook():
    """concourse's trace path imports antenv.axon_hooks, which this image's
    antenv lacks; register an equivalent shim so tracing works."""
    import sys
    try:
        import antenv.axon_hooks  # noqa: F401
        return
    except ImportError:
        pass
    import types
    mod = types.ModuleType("antenv.axon_hooks")
    hook = [None]
    mod.set_axon_ntff_profile_hook = lambda h: hook.__setitem__(0, h)
    mod.get_axon_ntff_profile_hook = lambda: hook[0]
    sys.modules["antenv.axon_hooks"] = mod
    try:
        from trn_agent_boot.trn_boot import _ntff_profile_via_ctypes
        mod.set_axon_ntff_profile_hook(
            _ntff_profile_via_ctypes("/opt/axon/libaxon_pjrt.so"))
    except Exception:
        pass


def run(queries, keys, values, trace=False, tmpdir=None):
    """Run on 8 cores; returns (output, BassKernelResults)."""
    _ensure_ntff_hook()
    from concourse.bass_utils import run_bass_kernel_spmd

    nc = _get_nc()
    queries = np.ascontiguousarray(queries, dtype=np.float32)
    keys = np.ascontiguousarray(keys, dtype=np.float32)
    values = np.ascontiguousarray(values, dtype=np.float32)
    in_maps = []
    for c in range(N_CORES):
        s = slice(c * B_LOC, (c + 1) * B_LOC)
        in_maps.append({
            "queries": queries[s],
            "keys": keys[s],
            "values": values[s],
        })
    res = run_bass_kernel_spmd(nc, in_maps, core_ids=list(range(N_CORES)),
                               trace=trace, tmpdir=tmpdir)
    out = np.concatenate([res.results[c]["out"] for c in range(N_CORES)], axis=0)
    return out, res


def kernel(queries, keys, values):
    out, _ = run(queries, keys, values)
    return out


# revision 7
# speedup vs baseline: 1.3620x; 1.3620x over previous
"""Trainium2 Bass kernel for batched attention.

Problem: b=16 batches of softmax(Q K^T / sqrt(128)) V with n=m=2048, d=dv=128,
fp32 inputs/outputs.  Sharding: batch dim across 8 NeuronCores (2 per core).

v3 design (v1 baseline ~99us):
  - exp split across TWO engines: ACT does exact exp (fused 1/T scale); DVE
    does a Schraudolph int16 exp approximation in one tensor_scalar:
      m = convert_i16(S * 1024*log2(e)/T + (15360 - C_ADJ)); bitcast(m) ~ p.
    The approximation's mean bias cancels in softmax; its mantissa ripple
    (~2%) enters at sqrt(share) weight -> rel err ~1.5e-2 (tol 2e-2).
  - PE transposes write 8-to-a-PSUM-bank; one batched [128,1024] copy per
    bank moves them to SBUF (alternating ACT/DVE).
  - MM2: one PSUM accumulation group per output tile (16 chunk matmuls),
    denominator via ones column of [V|1]; DVE reciprocal + DVE per-partition
    scale; stores per 4 tiles on sync HWDGE.
  - PE work ~62us at 2.4GHz is the target critical path; ACT/DVE each ~53us.
"""

import math

import numpy as np

B = 16
N_CORES = 8
B_LOC = B // N_CORES  # 2 batches per core
N = 2048
M = 2048
D = 128
NT = N // 128  # 16
MT = M // 128  # 16
TEMP = 11.313708498984761
INV_TEMP = 1.0 / TEMP

ALPHA = 1024.0 * math.log2(math.e) / TEMP
C_ADJ = 38.0
BETA = 15360.0 - C_ADJ

# exp unit (c, h), production order i = 2*c + h; 12/32 units per batch on DVE.
N_UNITS = 2 * MT
DVE_SHARE_NUM = 12
DVE_UNIT = [((i * DVE_SHARE_NUM) % N_UNITS) < DVE_SHARE_NUM for i in range(N_UNITS)]

_CACHE = {}


def _build():
    import concourse.bacc as bacc
    import concourse.mybir as mybir
    import concourse.tile as tile
    from concourse.masks import make_identity

    f32 = mybir.dt.float32
    f16 = mybir.dt.float16
    i16 = mybir.dt.int16

    nc = bacc.Bacc("TRN2", target_bir_lowering=False, debug=False,
                   num_devices=N_CORES)
    q_dram = nc.dram_tensor("queries", [B_LOC, N, D], f32, kind="ExternalInput")
    k_dram = nc.dram_tensor("keys", [B_LOC, M, D], f32, kind="ExternalInput")
    v_dram = nc.dram_tensor("values", [B_LOC, M, D], f32, kind="ExternalInput")
    o_dram = nc.dram_tensor("out", [B_LOC, N, D], f32, kind="ExternalOutput")

    with tile.TileContext(nc) as tc:
        with (
            tc.tile_pool(name="const", bufs=1) as const_pool,
            tc.tile_pool(name="nat", bufs=4) as nat_pool,
            tc.tile_pool(name="qT", bufs=2) as qT_pool,
            tc.tile_pool(name="kT", bufs=2) as kT_pool,
            tc.tile_pool(name="vo", bufs=2) as vo_pool,
            tc.tile_pool(name="pT", bufs=26) as pT_pool,
            tc.tile_pool(name="oall", bufs=2) as o_pool,
            tc.tile_pool(name="small", bufs=8) as small_pool,
            tc.tile_pool(name="psS", bufs=3, space="PSUM") as psS_pool,
            tc.tile_pool(name="psX", bufs=2, space="PSUM") as psX_pool,
        ):
            ident = const_pool.tile([128, 128], f16)
            make_identity(nc, ident[:])

            # ---- tiles ------------------------------------------------
            q_nats, k_nats, vos, qTs, kTs, o_alls = [], [], [], [], [], []
            for b in range(B_LOC):
                q_nats.append(nat_pool.tile([128, N], f16, name="qnat", tag="nat"))
                k_nats.append(nat_pool.tile([128, M], f16, name="knat", tag="nat"))
                vos.append(vo_pool.tile([128, MT * 129], f16, name="vo", tag="vo"))
                qTs.append(qT_pool.tile([128, N], f16, name="qT", tag="qT"))
                kTs.append(kT_pool.tile([128, M], f16, name="kT", tag="kT"))
                o_alls.append(o_pool.tile([128, NT * 128], f32, name="oall",
                                          tag="oall"))
            pTs = {b: [] for b in range(B_LOC)}

            # ---- DMA helpers (SWDGE cast loads on Pool) ----------------
            def load_grp(dst, srcd, b, g):
                cs = slice(g * 4, (g + 1) * 4)
                nc.gpsimd.dma_start(
                    dst[:].rearrange("p (c d) -> p c d", d=128)[:, cs],
                    srcd[b].rearrange("(c p) d -> p c d", p=128)[:, cs])

            def load_v(b):
                vo = vos[b]
                nc.gpsimd.dma_start(
                    vo[:].rearrange("p (c w) -> p c w", w=129)[:, :, 0:128],
                    v_dram[b].rearrange("(c p) d -> p c d", p=128))
                nc.gpsimd.memset(
                    vo[:].rearrange("p (c w) -> p c w", w=129)[:, :, 128:129],
                    1.0)

            # ---- PE transpose of 8 chunks into one PSUM bank, 1 copy ---
            # eng alternates ACT/DVE for the batched copy.
            def tr_bank(dst, src, half, eng):
                pst = psX_pool.tile([128, 512], f32, name="pst", tag="psX")
                for g in range(8):
                    c = half * 8 + g
                    nc.tensor.transpose(
                        pst[:, g * 64:(g + 1) * 64].bitcast(f16),
                        src[:, c * 128:(c + 1) * 128], ident[:])
                cols = slice(half * 1024, (half + 1) * 1024)
                if eng == "act":
                    nc.scalar.activation(
                        dst[:, cols], pst[:].bitcast(f16),
                        mybir.ActivationFunctionType.Copy)
                else:
                    nc.vector.tensor_copy(dst[:, cols], pst[:].bitcast(f16))

            # ---- MM1 chunk + exp units ---------------------------------
            def mm1_exp(b, c):
                pT = pT_pool.tile([128, N], f16, tag="pT")
                pTs[b].append(pT)
                kT, qT = kTs[b], qTs[b]
                for h in range(2):
                    psS = psS_pool.tile([128, 1024], f32, tag="psS")
                    for j in range(2):
                        nc.tensor.matmul(
                            psS[:, j * 512:(j + 1) * 512],
                            kT[:, c * 128:(c + 1) * 128],
                            qT[:, h * 1024 + j * 512:h * 1024 + (j + 1) * 512],
                            start=True, stop=True)
                    cols = slice(h * 1024, (h + 1) * 1024)
                    if DVE_UNIT[2 * c + h]:
                        nc.vector.tensor_scalar(
                            pT[:, cols].bitcast(i16), psS[:],
                            ALPHA, BETA,
                            mybir.AluOpType.mult, mybir.AluOpType.add)
                    else:
                        nc.scalar.activation(
                            pT[:, cols], psS[:],
                            mybir.ActivationFunctionType.Exp, scale=INV_TEMP)

            # ---- MM2 single-group per tile + recip/norm/store ----------
            def mm2(b, t):
                psO = psX_pool.tile([128, 129], f32, name="psO", tag="psX")
                vo = vos[b]
                for c in range(MT):
                    nc.tensor.matmul(
                        psO[:],
                        pTs[b][c][:, t * 128:(t + 1) * 128],
                        vo[:, c * 129:(c + 1) * 129],
                        start=(c == 0), stop=(c == MT - 1))
                recip = small_pool.tile([128, 1], f32, tag="recip")
                nc.vector.reciprocal(recip[:], psO[:, 128:129])
                nc.vector.tensor_scalar(
                    o_alls[b][:, t * 128:(t + 1) * 128], psO[:, 0:128],
                    recip[:], None, mybir.AluOpType.mult)
                if t % 4 == 3:
                    g = t // 4
                    cs = slice(g * 4, (g + 1) * 4)
                    nc.sync.dma_start(
                        o_dram[b].rearrange("(c p) d -> p c d", p=128)[:, cs],
                        o_alls[b][:].rearrange("p (c d) -> p c d", d=128)[:, cs])

            # ================= program ==================================
            # input loads, batch 0: K chunks first (stationaries), then Q.
            load_grp(k_nats[0], k_dram, 0, 0)
            load_grp(k_nats[0], k_dram, 0, 1)
            load_grp(q_nats[0], q_dram, 0, 0)
            load_grp(q_nats[0], q_dram, 0, 1)
            load_grp(q_nats[0], q_dram, 0, 2)
            load_grp(q_nats[0], q_dram, 0, 3)
            load_grp(k_nats[0], k_dram, 0, 2)
            load_grp(k_nats[0], k_dram, 0, 3)
            load_v(0)
            for g in range(4):
                load_grp(k_nats[1], k_dram, 1, g)
                load_grp(q_nats[1], q_dram, 1, g)
            load_v(1)

            # batch-0 transposes; K first so MM1 c0 can go once Q lands.
            tr_bank(kTs[0], k_nats[0], 0, "act")
            tr_bank(qTs[0], q_nats[0], 0, "dve")
            tr_bank(qTs[0], q_nats[0], 1, "act")
            mm1_exp(0, 0)
            tr_bank(kTs[0], k_nats[0], 1, "dve")
            mm1_exp(0, 1)

            # batch-0 main stream; b1 transposes slotted mid-stream.
            for c in range(2, MT):
                mm1_exp(0, c)
                if c == 11:
                    tr_bank(kTs[1], k_nats[1], 0, "act")
                elif c == 12:
                    tr_bank(qTs[1], q_nats[1], 0, "dve")
                elif c == 13:
                    tr_bank(qTs[1], q_nats[1], 1, "act")
                elif c == 14:
                    tr_bank(kTs[1], k_nats[1], 1, "dve")

            # batch-1 stream with batch-0 MM2 interleaved.
            for c in range(MT):
                mm1_exp(1, c)
                mm2(0, c)

            # drain: batch-1 MM2.
            for t in range(NT):
                mm2(1, t)

    nc.compile()
    return nc


def _get_nc():
    if "nc" not in _CACHE:
        _CACHE["nc"] = _build()
    return _CACHE["nc"]


def _ensure_ntff_hook():
    """concourse's trace path imports antenv.axon_hooks, which this image's
    antenv lacks; register an equivalent shim so tracing works."""
    import sys
    try:
        import antenv.axon_hooks  # noqa: F401
        return
    except ImportError:
        pass
    import types
    mod = types.ModuleType("antenv.axon_hooks")
    hook = [None]
    mod.set_axon_ntff_profile_hook = lambda h: hook.__setitem__(0, h)
    mod.get_axon_ntff_profile_hook = lambda: hook[0]
    sys.modules["antenv.axon_hooks"] = mod
    try:
        from trn_agent_boot.trn_boot import _ntff_profile_via_ctypes
        mod.set_axon_ntff_profile_hook(
            _ntff_profile_via_ctypes("/opt/axon/libaxon_pjrt.so"))
    except Exception:
        pass


def run(queries, keys, values, trace=False, tmpdir=None):
    """Run on 8 cores; returns (output, BassKernelResults)."""
    _ensure_ntff_hook()
    from concourse.bass_utils import run_bass_kernel_spmd

    nc = _get_nc()
    queries = np.ascontiguousarray(queries, dtype=np.float32)
    keys = np.ascontiguousarray(keys, dtype=np.float32)
    values = np.ascontiguousarray(values, dtype=np.float32)
    in_maps = []
    for c in range(N_CORES):
        s = slice(c * B_LOC, (c + 1) * B_LOC)
        in_maps.append({
            "queries": queries[s],
            "keys": keys[s],
            "values": values[s],
        })
    res = run_bass_kernel_spmd(nc, in_maps, core_ids=list(range(N_CORES)),
                               trace=trace, tmpdir=tmpdir)
    out = np.concatenate([res.results[c]["out"] for c in range(N_CORES)], axis=0)
    return out, res


def kernel(queries, keys, values):
    out, _ = run(queries, keys, values)
    return out


# revision 8
# speedup vs baseline: 1.4717x; 1.0805x over previous
"""Trainium2 Bass kernel for batched attention.

Problem: b=16 batches of softmax(Q K^T / sqrt(128)) V with n=m=2048, d=dv=128,
fp32 inputs/outputs.  Sharding: batch dim across 8 NeuronCores (2 per core).

v3.1 design (v1 baseline ~99us, v3 ~96.6us):
  - exp split across TWO engines: ACT does exact exp (fused 1/T scale); DVE
    does a Schraudolph int16 exp approximation in one tensor_scalar
    (round-to-nearest convert verified on HW):
      m = rint(S * 1024*log2(e)/T + (15360 - C_ADJ)); bitcast(m) ~ p.
    Mean bias cancels in softmax; mantissa ripple (~1.8%) enters at
    sqrt(share); per-batch shares tuned with an exact numpy replica of the
    pipeline (validated to 4 digits against HW).
  - batch-0 Q/K transposed on PE (latency-critical), 8 to a PSUM bank, one
    batched [128,1024] copy per bank (ACT/DVE alternating). batch-1
    transposed by the DMA XBAR (dma_start_transpose) fully off-critical.
  - MM2 per batch in A/B halves: A = chunks 0..7 -> PSUM -> parked f16
    partial (keeps PE busy inside the same batch's exp window); B = chunks
    8..15 + PE identity-matmul merge of the parked partial into the same
    accumulation group. Ones column of [V|1] gives the denominator.
  - DVE reciprocal; normalize on DVE (phase B) / alternating DVE+ACT (drain);
    drain double-buffers PSUM via the psS pool (idle during the drain).
"""

import math

import numpy as np

B = 16
N_CORES = 8
B_LOC = B // N_CORES  # 2 batches per core
N = 2048
M = 2048
D = 128
NT = N // 128  # 16
MT = M // 128  # 16
TEMP = 11.313708498984761
INV_TEMP = 1.0 / TEMP

ALPHA = 1024.0 * math.log2(math.e) / TEMP
C_ADJ = 50.0
BETA = 15360.0 - C_ADJ

# exp unit (c, h) -> engine, production order i = 2*c + h, per batch parity.
N_UNITS = 2 * MT


def _unit_table(share_num):
    return [((i * share_num) % N_UNITS) < share_num for i in range(N_UNITS)]


DVE_UNITS = {0: _unit_table(14), 1: _unit_table(12)}

_CACHE = {}


def _build():
    import concourse.bacc as bacc
    import concourse.mybir as mybir
    import concourse.tile as tile
    from concourse.masks import make_identity

    f32 = mybir.dt.float32
    f16 = mybir.dt.float16
    i16 = mybir.dt.int16

    nc = bacc.Bacc("TRN2", target_bir_lowering=False, debug=False,
                   num_devices=N_CORES)
    q_dram = nc.dram_tensor("queries", [B_LOC, N, D], f32, kind="ExternalInput")
    k_dram = nc.dram_tensor("keys", [B_LOC, M, D], f32, kind="ExternalInput")
    v_dram = nc.dram_tensor("values", [B_LOC, M, D], f32, kind="ExternalInput")
    o_dram = nc.dram_tensor("out", [B_LOC, N, D], f32, kind="ExternalOutput")

    with tile.TileContext(nc) as tc:
        with (
            tc.tile_pool(name="const", bufs=1) as const_pool,
            tc.tile_pool(name="nat", bufs=4) as nat_pool,
            tc.tile_pool(name="qT", bufs=2) as qT_pool,
            tc.tile_pool(name="kT", bufs=2) as kT_pool,
            tc.tile_pool(name="vo", bufs=2) as vo_pool,
            tc.tile_pool(name="pT", bufs=26) as pT_pool,
            tc.tile_pool(name="oall", bufs=2) as o_pool,
            tc.tile_pool(name="pa", bufs=20) as pa_pool,
            tc.tile_pool(name="small", bufs=8) as small_pool,
            tc.tile_pool(name="psS", bufs=3, space="PSUM") as psS_pool,
            tc.tile_pool(name="psX", bufs=2, space="PSUM") as psX_pool,
        ):
            # ---- tiles ------------------------------------------------
            q_nats, k_nats, vos, qTs, kTs, o_alls = [], [], [], [], [], []
            for b in range(B_LOC):
                q_nats.append(nat_pool.tile([128, N], f16, name="qnat", tag="nat"))
                k_nats.append(nat_pool.tile([128, M], f16, name="knat", tag="nat"))
                vos.append(vo_pool.tile([128, MT * 129], f16, name="vo", tag="vo"))
                qTs.append(qT_pool.tile([128, N], f16, name="qT", tag="qT"))
                kTs.append(kT_pool.tile([128, M], f16, name="kT", tag="kT"))
                o_alls.append(o_pool.tile([128, NT * 128], f32, name="oall",
                                          tag="oall"))
            ident = const_pool.tile([128, 128], f16)
            pTs = {b: [] for b in range(B_LOC)}
            pas = {}

            # ---- DMA helpers (SWDGE cast loads on Pool) ----------------
            def load_grp(dst, srcd, b, g):
                cs = slice(g * 4, (g + 1) * 4)
                nc.gpsimd.dma_start(
                    dst[:].rearrange("p (c d) -> p c d", d=128)[:, cs],
                    srcd[b].rearrange("(c p) d -> p c d", p=128)[:, cs])

            def load_v(b):
                vo = vos[b]
                nc.gpsimd.dma_start(
                    vo[:].rearrange("p (c w) -> p c w", w=129)[:, :, 0:128],
                    v_dram[b].rearrange("(c p) d -> p c d", p=128))
                nc.gpsimd.memset(
                    vo[:].rearrange("p (c w) -> p c w", w=129)[:, :, 128:129],
                    1.0)

            # ---- PE transpose of 8 chunks into one PSUM bank, 1 copy ---
            def tr_bank(dst, src, half, eng):
                pst = psX_pool.tile([128, 512], f32, name="pst", tag="psX")
                for g in range(8):
                    c = half * 8 + g
                    nc.tensor.transpose(
                        pst[:, g * 64:(g + 1) * 64].bitcast(f16),
                        src[:, c * 128:(c + 1) * 128], ident[:])
                cols = slice(half * 1024, (half + 1) * 1024)
                if eng == "act":
                    nc.scalar.activation(
                        dst[:, cols], pst[:].bitcast(f16),
                        mybir.ActivationFunctionType.Copy)
                else:
                    nc.vector.tensor_copy(dst[:, cols], pst[:].bitcast(f16))

            def xbar(dst, src, half):
                cols = slice(half * 1024, (half + 1) * 1024)
                nc.sync.dma_start_transpose(
                    dst[:, cols].rearrange("k (c n) -> k c n", n=128),
                    src[:, cols])

            # ---- MM1 chunk + exp units ---------------------------------
            def mm1_exp(b, c):
                pT = pT_pool.tile([128, N], f16, tag="pT")
                pTs[b].append(pT)
                kT, qT = kTs[b], qTs[b]
                for h in range(2):
                    psS = psS_pool.tile([128, 1024], f32, tag="psS")
                    for j in range(2):
                        nc.tensor.matmul(
                            psS[:, j * 512:(j + 1) * 512],
                            kT[:, c * 128:(c + 1) * 128],
                            qT[:, h * 1024 + j * 512:h * 1024 + (j + 1) * 512],
                            start=True, stop=True)
                    cols = slice(h * 1024, (h + 1) * 1024)
                    if DVE_UNITS[b][2 * c + h]:
                        nc.vector.tensor_scalar(
                            pT[:, cols].bitcast(i16), psS[:],
                            ALPHA, BETA,
                            mybir.AluOpType.mult, mybir.AluOpType.add)
                    else:
                        nc.scalar.activation(
                            pT[:, cols], psS[:],
                            mybir.ActivationFunctionType.Exp, scale=INV_TEMP)

            # ---- MM2 halves -------------------------------------------
            def mm2_a(b, t):
                """Chunks 0..7 -> PSUM -> parked f16 partial in SBUF."""
                psA = psX_pool.tile([128, 129], f32, name="psA", tag="psX")
                vo = vos[b]
                for c in range(8):
                    nc.tensor.matmul(
                        psA[:],
                        pTs[b][c][:, t * 128:(t + 1) * 128],
                        vo[:, c * 129:(c + 1) * 129],
                        start=(c == 0), stop=(c == 7))
                pa = pa_pool.tile([128, 129], f16, name="pa", tag="pa")
                pas[(b, t)] = pa
                nc.vector.tensor_copy(pa[:], psA[:])

            def mm2_b(b, t, use_psS=False, norm_eng="dve", store_grp=4):
                """Chunks 8..15 + identity-merge of the parked partial,
                reciprocal + normalize + store."""
                pool = psS_pool if use_psS else psX_pool
                psO = pool.tile([128, 129], f32, name="psO",
                                tag="psS" if use_psS else "psX")
                vo = vos[b]
                for c in range(8, MT):
                    nc.tensor.matmul(
                        psO[:],
                        pTs[b][c][:, t * 128:(t + 1) * 128],
                        vo[:, c * 129:(c + 1) * 129],
                        start=(c == 8), stop=False)
                nc.tensor.matmul(
                    psO[:], ident[:], pas[(b, t)][:],
                    start=False, stop=True)
                recip = small_pool.tile([128, 1], f32, tag="recip")
                nc.vector.reciprocal(recip[:], psO[:, 128:129])
                dst = o_alls[b][:, t * 128:(t + 1) * 128]
                if norm_eng == "act":
                    nc.scalar.activation(
                        dst, psO[:, 0:128],
                        mybir.ActivationFunctionType.Copy, scale=recip[:])
                else:
                    nc.vector.tensor_scalar(
                        dst, psO[:, 0:128], recip[:], None,
                        mybir.AluOpType.mult)
                if (t + 1) % store_grp == 0:
                    g0 = t + 1 - store_grp
                    cs = slice(g0, t + 1)
                    nc.sync.dma_start(
                        o_dram[b].rearrange("(c p) d -> p c d", p=128)[:, cs],
                        o_alls[b][:].rearrange("p (c d) -> p c d", d=128)[:, cs])

            # ================= program ==================================
            # batch-0 critical loads first, identity after, rest follow.
            load_grp(k_nats[0], k_dram, 0, 0)
            load_grp(k_nats[0], k_dram, 0, 1)
            load_grp(q_nats[0], q_dram, 0, 0)
            load_grp(q_nats[0], q_dram, 0, 1)
            make_identity(nc, ident[:])
            load_grp(q_nats[0], q_dram, 0, 2)
            load_grp(q_nats[0], q_dram, 0, 3)
            load_grp(k_nats[0], k_dram, 0, 2)
            load_grp(k_nats[0], k_dram, 0, 3)
            load_v(0)
            for g in range(4):
                load_grp(k_nats[1], k_dram, 1, g)
                load_grp(q_nats[1], q_dram, 1, g)
            load_v(1)

            # batch-1 transposes via DMA XBAR, off the critical path.
            for half in range(2):
                xbar(kTs[1], k_nats[1], half)
            for half in range(2):
                xbar(qTs[1], q_nats[1], half)

            # phase A: batch-0 transposes + MM1/exp + own MM2 A-halves.
            tr_bank(kTs[0], k_nats[0], 0, "act")
            tr_bank(qTs[0], q_nats[0], 0, "dve")
            tr_bank(qTs[0], q_nats[0], 1, "act")
            mm1_exp(0, 0)
            tr_bank(kTs[0], k_nats[0], 1, "dve")
            mm1_exp(0, 1)
            for c in range(2, MT):
                mm1_exp(0, c)
                if c >= 8:
                    mm2_a(0, 2 * (c - 8))
                    mm2_a(0, 2 * (c - 8) + 1)

            # phase B: batch-1 MM1/exp + batch-0 B-halves + batch-1 A-halves.
            for c in range(MT):
                mm1_exp(1, c)
                mm2_b(0, c, use_psS=False, norm_eng="dve")
                if c >= 8:
                    mm2_a(1, 2 * (c - 8))
                    mm2_a(1, 2 * (c - 8) + 1)

            # phase C: batch-1 B-halves, pipelined across psX+psS banks.
            for t in range(NT):
                mm2_b(1, t, use_psS=(t % 2 == 1),
                      norm_eng=("act" if t % 2 == 0 else "dve"),
                      store_grp=(4 if t < 12 else 2))

    nc.compile()
    return nc


def _get_nc():
    if "nc" not in _CACHE:
        _CACHE["nc"] = _build()
    return _CACHE["nc"]


def _ensure_ntff_hook():
    """concourse's trace path imports antenv.axon_hooks, which this image's
    antenv lacks; register an equivalent shim so tracing works."""
    import sys
    try:
        import antenv.axon_hooks  # noqa: F401
        return
    except ImportError:
        pass
    import types
    mod = types.ModuleType("antenv.axon_hooks")
    hook = [None]
    mod.set_axon_ntff_profile_hook = lambda h: hook.__setitem__(0, h)
    mod.get_axon_ntff_profile_hook = lambda: hook[0]
    sys.modules["antenv.axon_hooks"] = mod
    try:
        from trn_agent_boot.trn_boot import _ntff_profile_via_ctypes
        mod.set_axon_ntff_profile_hook(
            _ntff_profile_via_ctypes("/opt/axon/libaxon_pjrt.so"))
    except Exception:
        pass


def run(queries, keys, values, trace=False, tmpdir=None):
    """Run on 8 cores; returns (output, BassKernelResults)."""
    _ensure_ntff_hook()
    from concourse.bass_utils import run_bass_kernel_spmd

    nc = _get_nc()
    queries = np.ascontiguousarray(queries, dtype=np.float32)
    keys = np.ascontiguousarray(keys, dtype=np.float32)
    values = np.ascontiguousarray(values, dtype=np.float32)
    in_maps = []
    for c in range(N_CORES):
        s = slice(c * B_LOC, (c + 1) * B_LOC)
        in_maps.append({
            "queries": queries[s],
            "keys": keys[s],
            "values": values[s],
        })
    res = run_bass_kernel_spmd(nc, in_maps, core_ids=list(range(N_CORES)),
                               trace=trace, tmpdir=tmpdir)
    out = np.concatenate([res.results[c]["out"] for c in range(N_CORES)], axis=0)
    return out, res


def kernel(queries, keys, values):
    out, _ = run(queries, keys, values)
    return out
